# revision 1
# baseline (speedup 1.0000x reference)
"""Causal self-attention (B=4, T=2048, C=1024, NH=16) on 8 TRN2 NeuronCores.

Sharding: core c -> batch b = c//2, head-group g = c%2 (8 heads, Dh=512).
Each core computes q/k/v projections for its head group on its batch,
fused causal attention (attT layout: k on partitions), and a partial
output projection through its row-slice of Wp. Host sums the two
partials per batch.

Device dataflow per core:
  xt [C,T] (host pre-transposed) --f32r--> qt/kt [128,4,T] (Dh on
  partitions, head pair per 128-chunk), v (bf16, per-k-chunk lhsT slots
  with a ones column that makes softmax denominators a free extra psum
  row of the AV matmul). Causal: only lower-triangle k-chunks computed;
  diagonal chunks masked with a host tri mask after exp. Normalization:
  collected s rows -> batch reciprocal -> PE broadcast matmul -> one
  tensor_tensor scale of yt. Out-proj: yt @ wp in f32r.

kernel(**inputs) takes the FULL unsharded inputs and returns the FULL
output. Self-contained: hardcodes all shapes, reads nothing from disk.
"""

import sys

sys.path.insert(0, "/opt/trn_rl_repo")

import numpy as np
import ml_dtypes
from contextlib import ExitStack

import concourse.bass as bass  # noqa: F401  (engine types referenced via nc)
import concourse.mybir as mybir
import concourse.tile as tile
from concourse import bacc
from concourse.bass_utils import run_bass_kernel_spmd

P = 128
B, T, C = 4, 2048, 1024
NH, HS = 16, 64
D = 512          # per-core head dim (8 heads)
H = 8            # local heads
f32 = mybir.dt.float32
f32r = mybir.dt.float32r
bf16 = mybir.dt.bfloat16
AFT = mybir.ActivationFunctionType


def build_nc(t=T, stage="full"):
    """Build the single-core SPMD program (same code, per-core data).

    stage: "A" (projections only), "B" (+attention), "full" — debug aid;
    partial stages dump intermediates into `out` instead of the result.
    """
    assert t % 512 == 0
    nq = t // 512    # q blocks of 512
    nkc = t // 128   # k chunks of 128
    nb = t // 512    # T blocks for projections
    nco = C // P     # C chunks (8)

    nc = bacc.Bacc("TRN2", target_bir_lowering=False, debug=False, num_devices=8)

    xt_d = nc.dram_tensor("xt", [C, t], f32, kind="ExternalInput")
    wq_d = nc.dram_tensor("wq", [C, D], f32, kind="ExternalInput")
    wk_d = nc.dram_tensor("wk", [C, D], f32, kind="ExternalInput")
    wv_d = nc.dram_tensor("wv", [C, D], f32, kind="ExternalInput")
    wp_d = nc.dram_tensor("wp", [D, C], bf16, kind="ExternalInput")
    tri_d = nc.dram_tensor("tri", [P, P], bf16, kind="ExternalInput")
    bcm_d = nc.dram_tensor("bcm", [P, P], f32, kind="ExternalInput")
    out_d = nc.dram_tensor("out", [t, C], f32, kind="ExternalOutput")

    xt_r = xt_d[:].bitcast(f32r).rearrange("(co p) t -> p co t", p=P)
    wq_r = wq_d[:].bitcast(f32r).rearrange("(co p) d -> p co d", p=P)
    wk_r = wk_d[:].bitcast(f32r).rearrange("(co p) d -> p co d", p=P)
    wv_r = wv_d[:].bitcast(f32r).rearrange("(co p) d -> p co d", p=P)
    wp_r = wp_d[:].rearrange("(dc p) c -> p dc c", p=P)
    out_r = out_d[:].rearrange("(tc p) c -> p tc c", p=P)

    with tile.TileContext(nc) as tc, ExitStack() as ctx, nc.allow_low_precision(
        reason="f32r/bf16 attention kernel"
    ):
        # ---- persistent pool: spans projection + attention phases ----
        perm = ctx.enter_context(tc.tile_pool(name="perm", bufs=1))
        psum = ctx.enter_context(tc.tile_pool(name="psum", bufs=2, space="PSUM"))

        qt_sb = perm.tile([P, 4, t], f32r)   # Dh on partitions, head pair/chunk
        kt_sb = perm.tile([P, 4, t], f32r)
        v_sb = perm.tile([P, nkc, H, P], bf16)  # per-chunk AV lhsT slots
        tri_sb = perm.tile([P, P], bf16)
        bcm_sb = perm.tile([P, P], f32)
        nc.sync.dma_start(tri_sb[:], tri_d[:])
        nc.sync.dma_start(bcm_sb[:], bcm_d[:])
        # zero v slots (junk cols would put NaNs in unread psum rows)
        nc.gpsimd.memset(v_sb[:], 0.0)
        # ones columns: even head -> col 64 (sum row 64); odd -> col 0 (row 0)
        v5 = v_sb[:].rearrange("p k (hp par) c -> p k hp par c", par=2)
        nc.gpsimd.memset(v5[:, :, :, 0, 64:65], 1.0)
        nc.gpsimd.memset(v5[:, :, :, 1, 0:1], 1.0)

        # ---- phase A: projections ----
        with tc.tile_pool(name="pha", bufs=1) as pha:
            wq_sb = pha.tile([P, nco, D], f32r, tag="wq")
            wk_sb = pha.tile([P, nco, D], f32r, tag="wk")
            wv_sb = pha.tile([P, nco, D], f32r, tag="wv")
            for co in range(nco):  # per-chunk so matmuls start early
                nc.sync.dma_start(wq_sb[:, co, :], wq_r[:, co, :])
                nc.sync.dma_start(wk_sb[:, co, :], wk_r[:, co, :])
                nc.sync.dma_start(wv_sb[:, co, :], wv_r[:, co, :])
            for ib in range(nb):
                xtb = pha.tile([P, nco, 512], f32r, tag="xtb", bufs=2)
                nc.sync.dma_start(xtb[:], xt_r[:, :, ib * 512 : (ib + 1) * 512])
                for m in range(4):  # qt/kt row chunks of Dh
                    psq = psum.tile([P, 512], f32, tag="pa")
                    for co in range(nco):
                        nc.tensor.matmul(
                            psq[:],
                            wq_sb[:, co, m * P : (m + 1) * P],
                            xtb[:, co, :],
                            start=(co == 0),
                            stop=(co == nco - 1),
                        )
                    nc.vector.tensor_copy(
                        out=qt_sb[:, m, ib * 512 : (ib + 1) * 512], in_=psq[:]
                    )
                    psk = psum.tile([P, 512], f32, tag="pb")
                    for co in range(nco):
                        nc.tensor.matmul(
                            psk[:],
                            wk_sb[:, co, m * P : (m + 1) * P],
                            xtb[:, co, :],
                            start=(co == 0),
                            stop=(co == nco - 1),
                        )
                    nc.vector.tensor_copy(
                        out=kt_sb[:, m, ib * 512 : (ib + 1) * 512], in_=psk[:]
                    )
                for t4 in range(4):  # v chunks of 128 rows within this block
                    kc = ib * 4 + t4
                    psv = psum.tile([P, 512], f32, tag="pc")
                    for co in range(nco):
                        nc.tensor.matmul(
                            psv[:],
                            xtb[:, co, t4 * P : (t4 + 1) * P],
                            wv_sb[:, co, :],
                            start=(co == 0),
                            stop=(co == nco - 1),
                        )
                    # scatter heads into lhsT slots: even -> cols 0:64 of
                    # slot (par 0), odd -> cols 64:128 (par 1)
                    src = psv[:].rearrange("p (hp par c) -> p hp par c", par=2, c=64)
                    nc.vector.tensor_copy(
                        out=v5[:, kc, :, 0, 0:64], in_=src[:, :, 0, :]
                    )
                    nc.vector.tensor_copy(
                        out=v5[:, kc, :, 1, 64:128], in_=src[:, :, 1, :]
                    )

        if stage == "A":
            w_ = min(C, t)
            nc.sync.dma_start(out_r[:, 0, 0:w_], qt_sb[:, 0, 0:w_].bitcast(f32))
            nc.sync.dma_start(out_r[:, 1, 0:w_], kt_sb[:, 0, 0:w_].bitcast(f32))
            with tc.tile_pool(name="dbg", bufs=1) as dbg:
                vf = dbg.tile([P, 8 * P], f32)
                nc.vector.tensor_copy(
                    out=vf[:], in_=v_sb[:, 0, :, :].rearrange("p h c -> p (h c)")
                )
                nc.sync.dma_start(out_r[:, 2, :], vf[:])

        # ---- phase B: attention + normalization; phase C: out-proj ----
        if stage != "A":
          with tc.tile_pool(name="phb", bufs=1) as phb:
            yt_sb = phb.tile([P, 4, t], bf16)
            s_sb = phb.tile([P, 4, t], f32)    # rows 64 (even s) / 0 (odd s)
            nc.gpsimd.memset(s_sb[:], 1.0)     # unused rows must be finite

            def emit_norm(hp):
                # PE broadcast of 1/s via bcm, then one scale TT per block.
                # Emitted one head-pair late so the PE's in-order stream
                # never stalls on the ACT ln/exp chain.
                for jb in range(t // 512):
                    q0 = jb * 512
                    rb = psum.tile([P, 512], f32, tag="pc", bufs=2)
                    nc.tensor.matmul(
                        rb[:], bcm_sb[:], s_sb[:, hp, q0 : q0 + 512],
                        start=True, stop=True,
                    )
                    nc.vector.tensor_mul(
                        out=yt_sb[:, hp, q0 : q0 + 512],
                        in0=yt_sb[:, hp, q0 : q0 + 512],
                        in1=rb[:],
                    )

            for hp in range(4):
                lo, hi = slice(0, 64), slice(64, 128)
                for jq in range(nq):
                    q0 = jq * 512
                    nk = (jq + 1) * 4
                    psyE = psum.tile([P, 512], f32, tag="pe", bufs=1)
                    psyO = psum.tile([P, 512], f32, tag="po", bufs=1)
                    prev = None  # software-pipelined AV emission
                    for kc in range(nk):
                        d = kc - jq * 4
                        off = 128 * d if d >= 0 else 0
                        attA = phb.tile([P, 512], bf16, tag="attA", bufs=6)
                        attB = phb.tile([P, 512], bf16, tag="attB", bufs=6)
                        for att, par, sl in ((attA, 0, lo), (attB, 1, hi)):
                            ps = psum.tile(
                                [P, 512], f32, tag=("pa" if par == 0 else "pb"),
                                bufs=2,
                            )
                            nc.tensor.matmul(
                                ps[:, off:512],
                                kt_sb[sl, hp, kc * P : (kc + 1) * P],
                                qt_sb[sl, hp, q0 + off : q0 + 512],
                                start=True,
                                stop=True,
                            )
                            if off > 0:
                                nc.gpsimd.memset(att[:, 0:off], 0.0)
                            nc.scalar.activation(
                                att[:, off:512], ps[:, off:512], AFT.Exp,
                                scale=0.125,
                            )
                            if d >= 0:
                                nc.vector.tensor_mul(
                                    out=att[:, off : off + P],
                                    in0=att[:, off : off + P],
                                    in1=tri_sb[:],
                                )
                        if prev is not None:
                            pkc, pA, pB = prev
                            nc.tensor.matmul(
                                psyE[:], v_sb[:, pkc, 2 * hp, :], pA[:],
                                start=(pkc == 0), stop=False,
                            )
                            nc.tensor.matmul(
                                psyO[:], v_sb[:, pkc, 2 * hp + 1, :], pB[:],
                                start=(pkc == 0), stop=False,
                            )
                        prev = (kc, attA, attB)
                    pkc, pA, pB = prev
                    nc.tensor.matmul(
                        psyE[:], v_sb[:, pkc, 2 * hp, :], pA[:],
                        start=(pkc == 0), stop=True,
                    )
                    nc.tensor.matmul(
                        psyO[:], v_sb[:, pkc, 2 * hp + 1, :], pB[:],
                        start=(pkc == 0), stop=True,
                    )
                    # yt (unnormalized) + s rows out of the psums
                    nc.vector.tensor_copy(
                        out=yt_sb[lo, hp, q0 : q0 + 512], in_=psyE[0:64, :]
                    )
                    nc.vector.tensor_copy(
                        out=yt_sb[hi, hp, q0 : q0 + 512], in_=psyO[64:128, :]
                    )
                    nc.vector.tensor_copy(
                        out=s_sb[64:65, hp, q0 : q0 + 512], in_=psyE[64:65, :]
                    )
                    nc.vector.tensor_copy(
                        out=s_sb[0:1, hp, q0 : q0 + 512], in_=psyO[0:1, :]
                    )
                # 1/s = exp(-ln s) on ACT (DVE reciprocal is an iterative
                # divide ~6 cyc/elem and stalls the tail; approx_fast
                # miscomputes on HW)
                for row in (slice(0, 1), slice(64, 65)):
                    nc.scalar.activation(
                        s_sb[row, hp, :], s_sb[row, hp, :], AFT.Ln
                    )
                    nc.scalar.activation(
                        s_sb[row, hp, :], s_sb[row, hp, :], AFT.Exp, scale=-1.0
                    )
                if hp > 0:
                    emit_norm(hp - 1)
            emit_norm(3)

            if stage == "B":
                w_ = min(C, t)
                for mm in range(4):
                    dbg_f = phb.tile([P, w_], f32, tag="dbgf", bufs=2)
                    nc.vector.tensor_copy(
                        out=dbg_f[:], in_=yt_sb[:, mm, 0:w_]
                    )
                    nc.sync.dma_start(out_r[:, mm, 0:w_], dbg_f[:])

            # ---- phase C: out = yt.T @ wp ----
            if stage == "full":
              with tc.tile_pool(name="phc", bufs=1) as phc:
                wp_sb = phc.tile([P, 4, C], bf16, tag="wp")
                nc.sync.dma_start(wp_sb[:], wp_r)
                for tcn in range(t // P):
                    ob = phc.tile([P, C], f32, tag="ob", bufs=2)
                    for n2 in range(C // 512):
                        pso = psum.tile([P, 512], f32, tag="pa")
                        for dc in range(4):
                            nc.tensor.matmul(
                                pso[:],
                                yt_sb[:, dc, tcn * P : (tcn + 1) * P],
                                wp_sb[:, dc, n2 * 512 : (n2 + 1) * 512],
                                start=(dc == 0),
                                stop=(dc == 3),
                            )
                        nc.vector.tensor_copy(
                            out=ob[:, n2 * 512 : (n2 + 1) * 512], in_=pso[:]
                        )
                    nc.sync.dma_start(out_r[:, tcn, :], ob[:])

    nc.finalize()
    return nc


_NC = None


def _get_nc():
    global _NC
    if _NC is None:
        _NC = build_nc()
    return _NC


def make_in_maps(x, Wk, Wq, Wv, Wp):
    x = np.asarray(x, dtype=np.float32)
    Wk = np.asarray(Wk, dtype=np.float32)
    Wq = np.asarray(Wq, dtype=np.float32)
    Wv = np.asarray(Wv, dtype=np.float32)
    Wp = np.asarray(Wp, dtype=np.float32)
    tri = np.triu(np.ones((P, P), np.float32)).astype(ml_dtypes.bfloat16)
    bcm = np.zeros((P, P), np.float32)
    bcm[0, 64:128] = 1.0   # odd head r (s at row 0) -> yt rows 64:128
    bcm[64, 0:64] = 1.0    # even head r (s at row 64) -> yt rows 0:64
    in_maps = []
    for c in range(8):
        b, g = c // 2, c % 2
        sl = slice(g * D, (g + 1) * D)
        in_maps.append({
            "xt": np.ascontiguousarray(x[b].T),
            "wq": np.ascontiguousarray(Wq[:, sl]),
            "wk": np.ascontiguousarray(Wk[:, sl]),
            "wv": np.ascontiguousarray(Wv[:, sl]),
            "wp": np.ascontiguousarray(Wp[sl, :]).astype(ml_dtypes.bfloat16),
            "tri": tri,
            "bcm": bcm,
        })
    return in_maps


def _run(x, Wk, Wq, Wv, Wp, trace=False):
    nc = _get_nc()
    in_maps = make_in_maps(x, Wk, Wq, Wv, Wp)
    res = run_bass_kernel_spmd(nc, in_maps, core_ids=list(range(8)), trace=trace)
    parts = [res.results[c]["out"] for c in range(8)]
    out = np.stack(
        [parts[2 * b] + parts[2 * b + 1] for b in range(B)], axis=0
    ).astype(np.float32)
    return out, res


def kernel(x, Wk, Wq, Wv, Wp):
    out, _ = _run(x, Wk, Wq, Wv, Wp, trace=False)
    return out



# revision 14
# speedup vs baseline: 1.1724x; 1.1724x over previous
"""Causal self-attention (B=4, T=2048, C=1024, NH=16) on 8 TRN2 NeuronCores.

Sharding: core c -> batch b = c//2, head-group g = c%2 (8 heads, Dh=512).
Each core computes q/k/v projections for its head group on its batch,
fused causal attention, and a partial output projection through its
row-slice of Wp.  Host sums the two partials per batch.

v2 layout (all bf16 matmul inputs, fp32 psum):
  - Softmax exp merged into [128, 2048] psum groups holding BOTH head
    parities x 2 k-chunks -> 4x fewer / 4x wider ACT instructions.
  - Causal masking via host-built tri masks multiplied AFTER exp
    (DVE/Pool), so the QK matmuls and exp run on full tiles with no
    per-chunk memset/off bookkeeping.
  - Softmax denominators gathered into one [32, 512] tile; ln/exp
    batched per q-block (8 rows at a time) instead of [1, 2048] strips.
  - 1/s broadcast back to 128 rows via tiny selection-mask matmuls.
  - Projection, attention, normalization and out-projection emission is
    software-pipelined: proj(ib+1)/outproj(jq-1) pieces are round-robin
    interleaved between attention groups so no engine sits idle between
    phases.
  - DMA order: first x block first, weights after, so the PE starts
    ~7us in instead of ~40us.

kernel(**inputs) takes the FULL unsharded inputs and returns the FULL
output.  Self-contained: hardcodes all shapes, reads nothing from disk.
"""

import sys

sys.path.insert(0, "/opt/trn_rl_repo")

import numpy as np
import ml_dtypes
from contextlib import ExitStack

import concourse.bass as bass  # noqa: F401
import concourse.mybir as mybir
import concourse.tile as tile
from concourse import bacc
from concourse.bass_utils import run_bass_kernel_spmd

P = 128
B, T, C = 4, 2048, 1024
NH, HS = 16, 64
D = 512          # per-core head dim (8 heads)
H = 8            # local heads
f32 = mybir.dt.float32
bf16 = mybir.dt.bfloat16
AFT = mybir.ActivationFunctionType


def build_nc(t=T):
    assert t % 512 == 0
    nq = t // 512     # q blocks of 512
    nkc = t // 128    # k chunks of 128
    nco = C // P      # contraction chunks (8)

    nc = bacc.Bacc("TRN2", target_bir_lowering=False, debug=False, num_devices=8)

    xt_d = nc.dram_tensor("xt", [C, t], bf16, kind="ExternalInput")
    wq_d = nc.dram_tensor("wq", [C, D], bf16, kind="ExternalInput")
    wk_d = nc.dram_tensor("wk", [C, D], bf16, kind="ExternalInput")
    wv_d = nc.dram_tensor("wv", [C, D], bf16, kind="ExternalInput")
    wp_d = nc.dram_tensor("wp", [D, C], bf16, kind="ExternalInput")
    dmask_d = nc.dram_tensor("dmask", [P, 2, 2048], bf16, kind="ExternalInput")
    selm_d = nc.dram_tensor("selm", [8, 4, P], bf16, kind="ExternalInput")
    out_d = nc.dram_tensor("out", [t, C], f32, kind="ExternalOutput")

    xt_r = xt_d[:].rearrange("(co p) t -> p co t", p=P)
    wq_r = wq_d[:].rearrange("(co p) d -> p co d", p=P)
    wk_r = wk_d[:].rearrange("(co p) d -> p co d", p=P)
    wv_r = wv_d[:].rearrange("(co p) d -> p co d", p=P)
    wp_r = wp_d[:].rearrange("(dc p) c -> p dc c", p=P)
    out_r = out_d[:].rearrange("(tc p) c -> p tc c", p=P)

    with tile.TileContext(nc) as tc, ExitStack() as ctx, nc.allow_low_precision(
        reason="bf16 attention kernel"
    ):
        sb = ctx.enter_context(tc.tile_pool(name="sb", bufs=1))
        psum = ctx.enter_context(tc.tile_pool(name="psum", bufs=1, space="PSUM"))

        qt_sb = sb.tile([P, 4, t], bf16)
        kt_sb = sb.tile([P, 4, t], bf16)
        v_sb = sb.tile([P, nkc, H, P], bf16)
        yt_sb = sb.tile([P, 4, t], bf16)
        # per-jq softmax-denominator tiles: row r = hp*2 + par, par0=odd head
        sg_sb = [
            sb.tile([8, 512], f32, tag=f"sg{j}", name=f"sg{j}") for j in range(nq)
        ]
        sgi_sb = [
            sb.tile([8, 512], bf16, tag=f"sgi{j}", name=f"sgi{j}")
            for j in range(nq)
        ]
        dmask_sb = sb.tile([P, 2, 2048], bf16)
        selm_sb = sb.tile([8, 4, P], bf16)
        wq_sb = sb.tile([P, nco, D], bf16)
        wk_sb = sb.tile([P, nco, D], bf16)
        wv_sb = sb.tile([P, nco, D], bf16)
        wp_sb = sb.tile([P, 4, C], bf16)

        v5 = v_sb[:].rearrange("p k (hp par) c -> p k hp par c", par=2)

        # ---------- prelude: DMAs + memsets ----------
        # xtb block 0 first so the first projection matmul starts early.
        xtb0 = sb.tile([P, nco, 512], bf16, tag="xtb", bufs=2)
        nc.sync.dma_start(xtb0[:], xt_r[:, :, 0:512])
        for co in range(nco):
            nc.sync.dma_start(wq_sb[:, co, :], wq_r[:, co, :])
            nc.sync.dma_start(wk_sb[:, co, :], wk_r[:, co, :])
            nc.sync.dma_start(wv_sb[:, co, :], wv_r[:, co, :])
        nc.sync.dma_start(dmask_sb[:], dmask_d[:])
        nc.sync.dma_start(selm_sb[:], selm_d[:])
        nc.sync.dma_start(wp_sb[:], wp_r)
        nc.gpsimd.memset(v_sb[:], 0.0)
        nc.gpsimd.memset(v5[:, :, :, 0, 64:65], 1.0)  # even head: s at row 64
        nc.gpsimd.memset(v5[:, :, :, 1, 0:1], 1.0)    # odd head: s at row 0
        for j in range(nq):
            nc.vector.memset(sg_sb[j][:], 1.0)
            nc.vector.memset(sgi_sb[j][:], 1.0)

        # ---------- emission units ----------
        def proj_units(ib):
            """Projection of x block ib -> qt/kt/v.  13 units."""
            if ib == 0:
                xtb = xtb0
            else:
                xtb = sb.tile([P, nco, 512], bf16, tag="xtb", bufs=2)
                nc.sync.dma_start(xtb[:], xt_r[:, :, ib * 512 : (ib + 1) * 512])
                yield
            for m in range(4):
                for w_sb, dst in ((wq_sb, qt_sb), (wk_sb, kt_sb)):
                    ps = psum.tile([P, 512], f32, tag="io", bufs=2)
                    for co in range(nco):
                        nc.tensor.matmul(
                            ps[:],
                            w_sb[:, co, m * P : (m + 1) * P],
                            xtb[:, co, :],
                            start=(co == 0),
                            stop=(co == nco - 1),
                        )
                    nc.vector.tensor_copy(
                        out=dst[:, m, ib * 512 : (ib + 1) * 512], in_=ps[:]
                    )
                    yield
            for t4 in range(4):
                kc = ib * 4 + t4
                ps = psum.tile([P, 512], f32, tag="io", bufs=2)
                for co in range(nco):
                    nc.tensor.matmul(
                        ps[:],
                        xtb[:, co, t4 * P : (t4 + 1) * P],
                        wv_sb[:, co, :],
                        start=(co == 0),
                        stop=(co == nco - 1),
                    )
                src = ps[:].rearrange("p (hp par c) -> p hp par c", par=2, c=64)
                nc.vector.tensor_copy(out=v5[:, kc, :, 0, 0:64], in_=src[:, :, 0, :])
                nc.vector.tensor_copy(
                    out=v5[:, kc, :, 1, 64:128], in_=src[:, :, 1, :]
                )
                yield

        def att_units(hp, jq):
            """Attention for head-pair hp on q block jq.

            Groups of 2 k-chunks x 2 parities -> one [P, 2048] psum and
            one exp.  AV trails the exp by one group.
            """
            q0 = jq * 512
            nk = 4 * (jq + 1)
            ng = 2 * (jq + 1)
            psyE = psum.tile([P, 512], f32, tag="ye", bufs=1)
            psyO = psum.tile([P, 512], f32, tag="yo", bufs=1)

            def av_emit(g, attg):
                for i in (0, 1):
                    kc = 2 * g + i
                    d = kc - 4 * jq
                    off = 128 * d if d >= 0 else 0
                    for par, psy in ((0, psyE), (1, psyO)):
                        col = i * 1024 + par * 512
                        nc.tensor.matmul(
                            psy[:, off:512],
                            v_sb[:, kc, 2 * hp + par, :],
                            attg[:, col + off : col + 512],
                            start=(kc == 0),
                            stop=(kc == nk - 1),
                            skip_group_check=True,
                        )

            prev = None
            for g in range(ng):
                qk = psum.tile([P, 2048], f32, tag="qk", bufs=1)
                for i in (0, 1):
                    kc = 2 * g + i
                    for par, sl in ((0, slice(0, 64)), (1, slice(64, 128))):
                        col = i * 1024 + par * 512
                        nc.tensor.matmul(
                            qk[:, col : col + 512],
                            kt_sb[sl, hp, kc * P : (kc + 1) * P],
                            qt_sb[sl, hp, q0 : q0 + 512],
                            start=True,
                            stop=True,
                        )
                attg = sb.tile([P, 2048], bf16, tag="att", bufs=3)
                nc.scalar.activation(attg[:], qk[:], AFT.Exp, scale=0.125)
                if g >= 2 * jq:  # diagonal group -> causal mask (Pool engine)
                    dv = g - 2 * jq
                    nc.gpsimd.tensor_mul(
                        out=attg[:], in0=attg[:], in1=dmask_sb[:, dv, :]
                    )
                if prev is not None:
                    av_emit(*prev)
                prev = (g, attg)
                yield
            av_emit(*prev)
            # drains: yt + s rows (staged, then a tiny DMA gathers psum
            # rows {0 (odd s), 64 (even s)} into adjacent sg rows)
            nc.vector.tensor_copy(
                out=yt_sb[0:64, hp, q0 : q0 + 512], in_=psyE[0:64, :]
            )
            nc.vector.tensor_copy(
                out=yt_sb[64:128, hp, q0 : q0 + 512], in_=psyO[64:128, :]
            )
            stmp = sb.tile([P, 512], f32, tag="stmp", bufs=2)
            nc.vector.tensor_copy(out=stmp[64:65, :], in_=psyE[64:65, :])
            nc.vector.tensor_copy(out=stmp[0:1, :], in_=psyO[0:1, :])
            nc.sync.dma_start(
                sg_sb[jq][hp * 2 : hp * 2 + 2, :], stmp[0:65:64, :]
            )
            yield

        def tail_units(jq):
            """Normalization + out-projection for q block jq."""
            q0 = jq * 512
            # 1/s = exp(-ln s), batched over the 8 (hp, par) rows
            nc.scalar.activation(sg_sb[jq][:], sg_sb[jq][:], AFT.Ln)
            nc.scalar.activation(
                sgi_sb[jq][:], sg_sb[jq][:], AFT.Exp, scale=-1.0
            )
            yield
            for hp in range(4):
                rb = psum.tile([P, 512], f32, tag="io", bufs=2)
                nc.tensor.matmul(
                    rb[:],
                    selm_sb[:, hp, :],
                    sgi_sb[jq][:],
                    start=True,
                    stop=True,
                )
                nc.vector.tensor_mul(
                    out=yt_sb[:, hp, q0 : q0 + 512],
                    in0=yt_sb[:, hp, q0 : q0 + 512],
                    in1=rb[:],
                )
                yield
            for tcn in range(jq * 4, jq * 4 + 4):
                ob = sb.tile([P, C], f32, tag="ob", bufs=2)
                for n2 in (0, 1):
                    pso = psum.tile([P, 512], f32, tag="io", bufs=2)
                    for dc in range(4):
                        nc.tensor.matmul(
                            pso[:],
                            yt_sb[:, dc, tcn * P : (tcn + 1) * P],
                            wp_sb[:, dc, n2 * 512 : (n2 + 1) * 512],
                            start=(dc == 0),
                            stop=(dc == 3),
                        )
                    nc.vector.tensor_copy(
                        out=ob[:, n2 * 512 : (n2 + 1) * 512], in_=pso[:]
                    )
                nc.sync.dma_start(out_r[:, tcn, :], ob[:])
                yield

        def chain(*gens):
            for g in gens:
                yield from g

        def run_merged(main_gens, filler_gens, n_main, n_fill):
            main = chain(*main_gens)
            fill = chain(*filler_gens)
            ratio = (n_fill / n_main) if n_main else 0.0
            acc = 0.0
            done = object()
            fill_done = n_fill == 0
            for _ in main:
                acc += ratio
                while acc >= 1.0 and not fill_done:
                    fill_done = next(fill, done) is done
                    acc -= 1.0
            while not fill_done:
                fill_done = next(fill, done) is done

        # ---------- emission schedule ----------
        for _ in proj_units(0):
            pass
        for jq in range(nq):
            mains = [att_units(hp, jq) for hp in range(4)]
            n_main = 4 * (2 * (jq + 1) + 1)
            fillers = []
            n_fill = 0
            if jq + 1 < nq:
                fillers.append(proj_units(jq + 1))
                n_fill += 13
            if jq >= 1:
                fillers.append(tail_units(jq - 1))
                n_fill += 9
            run_merged(mains, fillers, n_main, n_fill)
        for _ in tail_units(nq - 1):
            pass

    nc.finalize()
    return nc


_NC = None


def _get_nc():
    global _NC
    if _NC is None:
        _NC = build_nc()
    return _NC


def make_in_maps(x, Wk, Wq, Wv, Wp, t=T):
    x = np.asarray(x, dtype=np.float32)
    Wk = np.asarray(Wk, dtype=np.float32)
    Wq = np.asarray(Wq, dtype=np.float32)
    Wv = np.asarray(Wv, dtype=np.float32)
    Wp = np.asarray(Wp, dtype=np.float32)
    bf = ml_dtypes.bfloat16

    # diag-group causal masks: group variant v covers chunk offsets d=2v+i,
    # col layout [E-i0 | O-i0 | E-i1 | O-i1]
    kk = np.arange(P)[:, None]
    qq = np.arange(512)[None, :]
    dmask = np.zeros((P, 2, 2048), np.float32)
    for v in range(2):
        for i in range(2):
            d = 2 * v + i
            blk = (qq >= d * P + kk).astype(np.float32)
            for par in range(2):
                b2 = i * 2 + par
                dmask[:, v, b2 * 512 : (b2 + 1) * 512] = blk
    dmask = dmask.astype(bf)

    # selection masks for broadcasting 1/s rows; sg row hp*2+0 holds the
    # ODD head's s (psum row 0), hp*2+1 the even head's (psum row 64)
    selm = np.zeros((8, 4, P), np.float32)
    for hp in range(4):
        selm[hp * 2 + 0, hp, 64:128] = 1.0
        selm[hp * 2 + 1, hp, 0:64] = 1.0
    selm = selm.astype(bf)

    in_maps = []
    for c in range(8):
        b, g = c // 2, c % 2
        sl = slice(g * D, (g + 1) * D)
        in_maps.append({
            "xt": np.ascontiguousarray(x[b, :t].T).astype(bf),
            "wq": np.ascontiguousarray(Wq[:, sl]).astype(bf),
            "wk": np.ascontiguousarray(Wk[:, sl]).astype(bf),
            "wv": np.ascontiguousarray(Wv[:, sl]).astype(bf),
            "wp": np.ascontiguousarray(Wp[sl, :]).astype(bf),
            "dmask": dmask,
            "selm": selm,
        })
    return in_maps


def _run(x, Wk, Wq, Wv, Wp, trace=False):
    nc = _get_nc()
    in_maps = make_in_maps(x, Wk, Wq, Wv, Wp)
    res = run_bass_kernel_spmd(nc, in_maps, core_ids=list(range(8)), trace=trace)
    parts = [res.results[c]["out"] for c in range(8)]
    out = np.stack(
        [parts[2 * b] + parts[2 * b + 1] for b in range(B)], axis=0
    ).astype(np.float32)
    return out, res


def kernel(x, Wk, Wq, Wv, Wp):
    out, _ = _run(x, Wk, Wq, Wv, Wp, trace=False)
    return out


# revision 24
# speedup vs baseline: 1.4133x; 1.2055x over previous
"""Causal self-attention (B=4, T=2048, C=1024, NH=16) on 8 TRN2 NeuronCores.

Sharding: core c -> batch b = c//2, head-group g = c%2 (8 heads, Dh=512).
Each core computes q/k/v projections for its head group on its batch,
fused causal attention, and a partial output projection through its
row-slice of Wp.  Host sums the two partials per batch.

v2 layout (all bf16 matmul inputs, fp32 psum):
  - Softmax exp merged into [128, 2048] psum groups holding BOTH head
    parities x 2 k-chunks -> 4x fewer / 4x wider ACT instructions.
  - Causal masking via host-built tri masks multiplied AFTER exp
    (DVE/Pool), so the QK matmuls and exp run on full tiles with no
    per-chunk memset/off bookkeeping.
  - Softmax denominators gathered into one [32, 512] tile; ln/exp
    batched per q-block (8 rows at a time) instead of [1, 2048] strips.
  - 1/s broadcast back to 128 rows via tiny selection-mask matmuls.
  - Projection, attention, normalization and out-projection emission is
    software-pipelined: proj(ib+1)/outproj(jq-1) pieces are round-robin
    interleaved between attention groups so no engine sits idle between
    phases.
  - DMA order: first x block first, weights after, so the PE starts
    ~7us in instead of ~40us.

kernel(**inputs) takes the FULL unsharded inputs and returns the FULL
output.  Self-contained: hardcodes all shapes, reads nothing from disk.
"""

import sys

sys.path.insert(0, "/opt/trn_rl_repo")

import numpy as np
import ml_dtypes
from contextlib import ExitStack

import concourse.bass as bass  # noqa: F401
import concourse.mybir as mybir
import concourse.tile as tile
from concourse import bacc
from concourse.bass_utils import run_bass_kernel_spmd

P = 128
B, T, C = 4, 2048, 1024
NH, HS = 16, 64
D = 512          # per-core head dim (8 heads)
H = 8            # local heads
f32 = mybir.dt.float32
bf16 = mybir.dt.bfloat16
AFT = mybir.ActivationFunctionType


def build_nc(t=T):
    assert t % 512 == 0
    nq = t // 512     # q blocks of 512
    nkc = t // 128    # k chunks of 128
    nco = C // P      # contraction chunks (8)

    nc = bacc.Bacc("TRN2", target_bir_lowering=False, debug=False, num_devices=8)

    xt_d = nc.dram_tensor("xt", [C, t], bf16, kind="ExternalInput")
    wq_d = nc.dram_tensor("wq", [C, D], bf16, kind="ExternalInput")
    wk_d = nc.dram_tensor("wk", [C, D], bf16, kind="ExternalInput")
    wv_d = nc.dram_tensor("wv", [C, D], bf16, kind="ExternalInput")
    wp_d = nc.dram_tensor("wp", [D, C], bf16, kind="ExternalInput")
    tri_d = nc.dram_tensor("tri", [P, P], bf16, kind="ExternalInput")
    selm_d = nc.dram_tensor("selm", [8, 4, P], bf16, kind="ExternalInput")
    out_d = nc.dram_tensor("out", [t, C], f32, kind="ExternalOutput")

    xt_r = xt_d[:].rearrange("(co p) t -> p co t", p=P)
    wq_r = wq_d[:].rearrange("(co p) d -> p co d", p=P)
    wk_r = wk_d[:].rearrange("(co p) d -> p co d", p=P)
    wv_r = wv_d[:].rearrange("(co p) d -> p co d", p=P)
    wp_r = wp_d[:].rearrange("(dc p) c -> p dc c", p=P)
    out_r = out_d[:].rearrange("(tc p) c -> p tc c", p=P)

    with tile.TileContext(nc) as tc, ExitStack() as ctx, nc.allow_low_precision(
        reason="bf16 attention kernel"
    ):
        sb = ctx.enter_context(tc.tile_pool(name="sb", bufs=1))
        psum = ctx.enter_context(tc.tile_pool(name="psum", bufs=1, space="PSUM"))

        qt_sb = sb.tile([P, 4, t], bf16)
        kt_sb = sb.tile([P, 4, t], bf16)
        v_sb = sb.tile([P, nkc, H, P], bf16)
        yt_sb = sb.tile([P, 4, t], bf16)
        # per-jq softmax-denominator tiles: row r = hp*2 + par, par0=odd head
        sg_sb = [
            sb.tile([8, 512], f32, tag=f"sg{j}", name=f"sg{j}") for j in range(nq)
        ]
        sgi_sb = [
            sb.tile([8, 512], bf16, tag=f"sgi{j}", name=f"sgi{j}")
            for j in range(nq)
        ]
        tri_sb = sb.tile([P, P], bf16)
        selm_sb = sb.tile([8, 4, P], bf16)
        wq_sb = sb.tile([P, nco, D], bf16)
        wk_sb = sb.tile([P, nco, D], bf16)
        wv_sb = sb.tile([P, nco, D], bf16)
        wp_sb = sb.tile([P, 4, C], bf16)

        v5 = v_sb[:].rearrange("p k (hp par) c -> p k hp par c", par=2)

        # ---------- prelude: DMAs + memsets ----------
        # xtb block 0 first so the first projection matmul starts early.
        xtb0 = sb.tile([P, nco, 512], bf16, tag="xtb", bufs=2)
        nc.sync.dma_start(xtb0[:], xt_r[:, :, 0:512])
        for co in range(nco):
            nc.sync.dma_start(wq_sb[:, co, :], wq_r[:, co, :])
            nc.sync.dma_start(wk_sb[:, co, :], wk_r[:, co, :])
            nc.sync.dma_start(wv_sb[:, co, :], wv_r[:, co, :])
        nc.sync.dma_start(tri_sb[:], tri_d[:])
        nc.sync.dma_start(selm_sb[:], selm_d[:])
        nc.sync.dma_start(wp_sb[:], wp_r)
        nc.gpsimd.memset(v_sb[:], 0.0)
        nc.gpsimd.memset(v5[:, :, :, 0, 64:65], 1.0)  # even head: s at row 64
        nc.gpsimd.memset(v5[:, :, :, 1, 0:1], 1.0)    # odd head: s at row 0
        for j in range(nq):
            nc.vector.memset(sg_sb[j][:], 1.0)
            nc.vector.memset(sgi_sb[j][:], 1.0)

        # ---------- emission units ----------
        def proj_units(ib):
            """Projection of x block ib -> qt/kt/v.  13 units."""
            if ib == 0:
                xtb = xtb0
            else:
                xtb = sb.tile([P, nco, 512], bf16, tag="xtb", bufs=2)
                nc.sync.dma_start(xtb[:], xt_r[:, :, ib * 512 : (ib + 1) * 512])
                yield
            for m in range(4):
                for w_sb, dst in ((wq_sb, qt_sb), (wk_sb, kt_sb)):
                    ps = psum.tile([P, 512], f32, tag="io", bufs=2)
                    for co in range(nco):
                        nc.tensor.matmul(
                            ps[:],
                            w_sb[:, co, m * P : (m + 1) * P],
                            xtb[:, co, :],
                            start=(co == 0),
                            stop=(co == nco - 1),
                        )
                    nc.vector.tensor_copy(
                        out=dst[:, m, ib * 512 : (ib + 1) * 512], in_=ps[:]
                    )
                    yield
            for t4 in range(4):
                kc = ib * 4 + t4
                ps = psum.tile([P, 512], f32, tag="io", bufs=2)
                for co in range(nco):
                    nc.tensor.matmul(
                        ps[:],
                        xtb[:, co, t4 * P : (t4 + 1) * P],
                        wv_sb[:, co, :],
                        start=(co == 0),
                        stop=(co == nco - 1),
                    )
                src = ps[:].rearrange("p (hp par c) -> p hp par c", par=2, c=64)
                nc.vector.tensor_copy(out=v5[:, kc, :, 0, 0:64], in_=src[:, :, 0, :])
                nc.vector.tensor_copy(
                    out=v5[:, kc, :, 1, 64:128], in_=src[:, :, 1, :]
                )
                yield

        def att_units(hp, jq):
            """Attention for head-pair hp on q block jq.

            One k-chunk per group: QK-E + QK-O into a double-buffered
            [P, 1024] psum, one exp over both parities.  Diagonal chunks
            get small memset + tri-strip masking (off the QK->exp chain).
            AV trails the exp by one chunk.
            """
            q0 = jq * 512
            nk = 4 * (jq + 1)
            psyE = psum.tile([P, 512], f32, tag="ye", bufs=1)
            psyO = psum.tile([P, 512], f32, tag="yo", bufs=1)

            def av_emit(kc, attg):
                d = kc - 4 * jq
                off = 128 * d if d >= 0 else 0
                for par, psy in ((0, psyE), (1, psyO)):
                    nc.tensor.matmul(
                        psy[:, off:512],
                        v_sb[:, kc, 2 * hp + par, :],
                        attg[:, par * 512 + off : par * 512 + 512],
                        start=(kc == 0),
                        stop=(kc == nk - 1),
                        skip_group_check=True,
                    )

            prev = None
            for kc in range(nk):
                d = kc - 4 * jq
                qk = psum.tile([P, 1024], f32, tag="qk", bufs=2)
                for par, sl in ((0, slice(0, 64)), (1, slice(64, 128))):
                    col = par * 512
                    nc.tensor.matmul(
                        qk[:, col : col + 512],
                        kt_sb[sl, hp, kc * P : (kc + 1) * P],
                        qt_sb[sl, hp, q0 : q0 + 512],
                        start=True,
                        stop=True,
                    )
                attg = sb.tile([P, 1024], bf16, tag="att", bufs=4)
                nc.scalar.activation(attg[:], qk[:], AFT.Exp, scale=0.125)
                if d >= 0:  # diagonal chunk -> causal tri mask on the
                    # boundary strip; cols below off are never read by AV
                    off = 128 * d
                    for par in (0, 1):
                        col = par * 512
                        eng = nc.vector if par == 0 else nc.gpsimd
                        eng.tensor_mul(
                            out=attg[:, col + off : col + off + P],
                            in0=attg[:, col + off : col + off + P],
                            in1=tri_sb[:],
                        )
                if prev is not None:
                    av_emit(*prev)
                prev = (kc, attg)
                yield
            av_emit(*prev)
            # drains: yt + s rows (staged, then a tiny DMA gathers psum
            # rows {0 (odd s), 64 (even s)} into adjacent sg rows)
            nc.vector.tensor_copy(
                out=yt_sb[0:64, hp, q0 : q0 + 512], in_=psyE[0:64, :]
            )
            nc.vector.tensor_copy(
                out=yt_sb[64:128, hp, q0 : q0 + 512], in_=psyO[64:128, :]
            )
            stmp = sb.tile([P, 512], f32, tag="stmp", bufs=2)
            nc.vector.tensor_copy(out=stmp[64:65, :], in_=psyE[64:65, :])
            nc.vector.tensor_copy(out=stmp[0:1, :], in_=psyO[0:1, :])
            nc.sync.dma_start(
                sg_sb[jq][hp * 2 : hp * 2 + 2, :], stmp[0:65:64, :]
            )
            yield

        def tail_units(jq):
            """Normalization + out-projection for q block jq."""
            q0 = jq * 512
            # 1/s on DVE (keeps the ACT table set pinned to exp)
            nc.vector.reciprocal(out=sgi_sb[jq][:], in_=sg_sb[jq][:])
            yield
            for hp in range(4):
                rb = psum.tile([P, 512], f32, tag="io", bufs=2)
                nc.tensor.matmul(
                    rb[:],
                    selm_sb[:, hp, :],
                    sgi_sb[jq][:],
                    start=True,
                    stop=True,
                )
                nc.vector.tensor_mul(
                    out=yt_sb[:, hp, q0 : q0 + 512],
                    in0=yt_sb[:, hp, q0 : q0 + 512],
                    in1=rb[:],
                )
                yield
            for tcn in range(jq * 4, jq * 4 + 4):
                ob = sb.tile([P, C], f32, tag="ob", bufs=2)
                for n2 in (0, 1):
                    pso = psum.tile([P, 512], f32, tag="io", bufs=2)
                    for dc in range(4):
                        nc.tensor.matmul(
                            pso[:],
                            yt_sb[:, dc, tcn * P : (tcn + 1) * P],
                            wp_sb[:, dc, n2 * 512 : (n2 + 1) * 512],
                            start=(dc == 0),
                            stop=(dc == 3),
                        )
                    nc.vector.tensor_copy(
                        out=ob[:, n2 * 512 : (n2 + 1) * 512], in_=pso[:]
                    )
                nc.sync.dma_start(out_r[:, tcn, :], ob[:])
                yield

        def chain(*gens):
            for g in gens:
                yield from g

        def run_merged(main_gens, filler_gens, n_main, n_fill):
            main = chain(*main_gens)
            fill = chain(*filler_gens)
            ratio = (n_fill / n_main) if n_main else 0.0
            acc = 0.0
            done = object()
            fill_done = n_fill == 0
            for _ in main:
                acc += ratio
                while acc >= 1.0 and not fill_done:
                    fill_done = next(fill, done) is done
                    acc -= 1.0
            while not fill_done:
                fill_done = next(fill, done) is done

        # ---------- emission schedule ----------
        for _ in proj_units(0):
            pass
        for jq in range(nq):
            mains = [att_units(hp, jq) for hp in range(4)]
            n_main = 4 * (4 * (jq + 1) + 1)
            fillers = []
            n_fill = 0
            if jq + 1 < nq:
                fillers.append(proj_units(jq + 1))
                n_fill += 13
            if jq >= 1:
                fillers.append(tail_units(jq - 1))
                n_fill += 9
            run_merged(mains, fillers, n_main, n_fill)
        for _ in tail_units(nq - 1):
            pass

    nc.finalize()
    return nc


_NC = None


def _get_nc():
    global _NC
    if _NC is None:
        _NC = build_nc()
    return _NC


def make_in_maps(x, Wk, Wq, Wv, Wp, t=T):
    x = np.asarray(x, dtype=np.float32)
    Wk = np.asarray(Wk, dtype=np.float32)
    Wq = np.asarray(Wq, dtype=np.float32)
    Wv = np.asarray(Wv, dtype=np.float32)
    Wp = np.asarray(Wp, dtype=np.float32)
    bf = ml_dtypes.bfloat16

    # lower-tri strip mask: tri[k, j] = 1 iff j >= k
    tri = np.triu(np.ones((P, P), np.float32)).astype(bf)

    # selection masks for broadcasting 1/s rows; sg row hp*2+0 holds the
    # ODD head's s (psum row 0), hp*2+1 the even head's (psum row 64)
    selm = np.zeros((8, 4, P), np.float32)
    for hp in range(4):
        selm[hp * 2 + 0, hp, 64:128] = 1.0
        selm[hp * 2 + 1, hp, 0:64] = 1.0
    selm = selm.astype(bf)

    in_maps = []
    for c in range(8):
        b, g = c // 2, c % 2
        sl = slice(g * D, (g + 1) * D)
        in_maps.append({
            "xt": np.ascontiguousarray(x[b, :t].T).astype(bf),
            "wq": np.ascontiguousarray(Wq[:, sl]).astype(bf),
            "wk": np.ascontiguousarray(Wk[:, sl]).astype(bf),
            "wv": np.ascontiguousarray(Wv[:, sl]).astype(bf),
            "wp": np.ascontiguousarray(Wp[sl, :]).astype(bf),
            "tri": tri,
            "selm": selm,
        })
    return in_maps


def _run(x, Wk, Wq, Wv, Wp, trace=False):
    nc = _get_nc()
    in_maps = make_in_maps(x, Wk, Wq, Wv, Wp)
    res = run_bass_kernel_spmd(nc, in_maps, core_ids=list(range(8)), trace=trace)
    parts = [res.results[c]["out"] for c in range(8)]
    out = np.stack(
        [parts[2 * b] + parts[2 * b + 1] for b in range(B)], axis=0
    ).astype(np.float32)
    return out, res


def kernel(x, Wk, Wq, Wv, Wp):
    out, _ = _run(x, Wk, Wq, Wv, Wp, trace=False)
    return out
